# revision 19
# baseline (speedup 1.0000x reference)
"""TRN2 Bass kernel for nn_MetrixSoftmax: softmax(-2 * ||x_b - w_o||_2, axis=o).

x: [8192, 256] f32, weight: [16384, 256] f32 -> out: [8192, 16384] f32.

Strategy: data-parallel shard x over batch across 8 cores (1024 rows each),
replicate weight; each core computes its full output rows so the softmax
needs no collectives. Per core (layout: partitions=batch rows, free=out):

  d2[b,o] = (x2[b]+256) + (w2[o]-256) - 2*x.w
  psum    = matmul-accumulated [-2*x.w + (w2-256)]  (w2 row folded in as a
            K=3 bf16 matmul of an exact 3-way bf16 split of w2-256)
  dist    = ACT Sqrt(psum + bias[b])   (bias = x2+256, per-partition AP)
  e       = ACT Exp(-2*dist), accum_out -> per-row partial sums
  out     = e * (1/sum)                (DVE reciprocal + tensor_scalar_mul)

Matmul modes:
  split7: x and w.T each split hi/lo bf16; 6 bf16 matmuls (hh, hl, lh) +
          w2 row -> ~7e-4 abs err on d2 (fp32-grade output).
  f32r3:  x, w.T fed as raw fp32 bits declared float32r (tf32-like 1-pass);
          2 matmuls + w2 row -> ~4e-2 abs err on d2 (~1e-3 output rel err).

Performance structure: chunks of G=4 psum banks [128, 2048]; matmuls run
product-major inside a group so the PE stationary operand is reused 4x;
ACT processes 2048-wide chunks; sqrt/exp phases are strictly ordered per
batch-tile to get exactly 2 ACT table loads per tile.
"""

import numpy as np
import ml_dtypes

B, IN, OUT = 8192, 256, 16384
NCORES = 8
BPC = B // NCORES     # 1024 batch rows per core
NT = BPC // 128       # 8 batch tiles of 128 rows
CH = 512              # matmul free-dim (one PSUM bank)
GRP = 4               # chunks per psum/slab group
GW = CH * GRP         # 2048 group width
NG = OUT // GW        # 8 groups per batch tile

MODE = "split7"       # "split7" (accurate) | "f32r3" | "fast" (fp16+bf16out)

_BF16 = ml_dtypes.bfloat16
_built = {}


def _bf16_split(a):
    hi = a.astype(_BF16)
    lo = (a - hi.astype(np.float32)).astype(_BF16)
    return hi, lo


def _build_fast():
    """fp16-matmul pipeline tuned for the ACT (scalar-engine) roofline.

    Per core: psum = [-2x.w + (w2-256)] via 2 fp16 K=128 matmuls + K=3 bf16
    w2 row (fp16 input quantization -> ~0.06 abs err on d2, ~3.5e-3 output
    scale_rel incl bf16 output rounding). ACT Sqrt reads PSUM directly with
    per-partition bias (x2+256) -- no DVE drain pass. ACT Exp (scale=-2)
    writes bf16 slabs 4096-wide with accum_out row sums. DVE does only the
    [128,4]->[128,1] sum reduce, reciprocal, and the 2x-mode bf16 normalize
    mul. Output DMA'd as bf16, upcast to f32 on host.

    Engine budget per core: ACT ~260us (bottleneck: sqrt 118 + exp 121 +
    16 table loads 20), PE ~170us, DVE ~50us, DMA ~42MB ~ 125us.
    """
    import concourse.bacc as bacc
    import concourse.tile as tile
    import concourse.mybir as mybir
    from concourse.tile import add_dep_helper

    F32 = mybir.dt.float32
    F16 = mybir.dt.float16
    BF16 = mybir.dt.bfloat16
    AF = mybir.ActivationFunctionType
    PW = 2 * GW           # 4096: exp/normalize/store pair width
    NP = OUT // PW        # 4 pairs per batch tile

    nc = bacc.Bacc("TRN2", target_bir_lowering=False, debug=False,
                   num_devices=NCORES)

    d_wt = nc.dram_tensor("wt", [IN, OUT], F16, kind="ExternalInput")
    d_xt = nc.dram_tensor("xt", [IN, BPC], F16, kind="ExternalInput")
    d_w2s = nc.dram_tensor("w2s", [3, OUT], BF16, kind="ExternalInput")
    d_x2b = nc.dram_tensor("x2b", [128, NT], F32, kind="ExternalInput")
    d_out = nc.dram_tensor("out", [BPC, OUT], BF16, kind="ExternalOutput")

    from contextlib import ExitStack
    with tile.TileContext(nc) as tc, ExitStack() as ctx:
        persist = ctx.enter_context(tc.tile_pool(name="persist", bufs=1))
        xt_pool = ctx.enter_context(tc.tile_pool(name="xtp", bufs=2))
        sf_pool = ctx.enter_context(tc.tile_pool(name="sfp", bufs=NP + 1))
        sb_pool = ctx.enter_context(tc.tile_pool(name="sbp", bufs=NP + 1))
        w2_pool = ctx.enter_context(tc.tile_pool(name="w2p", bufs=3))
        sums_pool = ctx.enter_context(tc.tile_pool(name="sumsp", bufs=2))
        psum_pool = ctx.enter_context(tc.tile_pool(name="psump", bufs=2, space="PSUM"))

        wh0 = persist.tile([128, OUT], F16, name="wh0")
        wh1 = persist.tile([128, OUT], F16, name="wh1")
        x2sb = persist.tile([128, NT], F32, name="x2sb")
        ones3 = persist.tile([3, 128], BF16, name="ones3")
        nc.vector.memset(ones3[:], 1.0)
        dummy_m = persist.tile([3, 512], BF16, name="dummy_m")
        nc.vector.memset(dummy_m[:], 0.0)

        x_tiles = {}

        def load_x(tt):
            ts_ = slice(tt * 128, (tt + 1) * 128)
            tiles = []
            for nm, p0 in (("xh0t", 0), ("xh1t", 128)):
                tl = xt_pool.tile([128, 128], F16, name=f"{nm}_{tt}", tag=nm)
                nc.sync.dma_start(tl[:], d_xt[p0:p0 + 128, ts_])
                tiles.append(tl)
            x_tiles[tt] = tiles

        # x/x2 first: they are tiny and the first matmul needs them; the 8.4MB
        # weight preload must not sit ahead of them in the Sync DMA queue.
        load_x(0)
        nc.sync.dma_start(x2sb[:], d_x2b[:, :])

        # ---- preload weights (column-split so early matmuls start early)
        NSPLIT = 8
        CW = OUT // NSPLIT
        for j in range(NSPLIT):
            cs = slice(j * CW, (j + 1) * CW)
            nc.sync.dma_start(wh0[:, cs], d_wt[0:128, cs])
            nc.sync.dma_start(wh1[:, cs], d_wt[128:256, cs])

        # ---- PE pre-heat: dummy matmuls keep the PE continuously busy while
        # the first weight/x DMAs land, so the p-state is fully ramped
        # (2.4 GHz) when real groups start; group 0's start=True resets psum.
        ps_warm = psum_pool.tile([128, GW], F32, name="ps_warm", tag="ps")
        for k in range(9):
            nc.tensor.matmul(ps_warm[:, 0:512], ones3[:, :], dummy_m[:],
                             start=(k == 0), stop=(k == 8))

        # w2 chunk tiles on the gpsimd SWDGE queue (separate from the
        # rec-gated output DMAs on the Sync queues).
        w2_tiles = {}

        def trig_w2(tt, gg):
            w2t = w2_pool.tile([3, GW], BF16, name=f"w2t_{tt}_{gg}", tag="w2t")
            nc.gpsimd.dma_start(w2t[:], d_w2s[:, gg * GW:(gg + 1) * GW])
            w2_tiles[(tt, gg)] = w2t

        # triple-buffered, triggered 2 groups ahead: the SWDGE trigger + DMA
        # latency (~2-3us) must never sit in the PE's path between groups.
        trig_w2(0, 0)
        trig_w2(0, 1)

        def flush_one(pending, p):
            # normalize (DVE 2x bf16) + store (Sync) ONE pair of the PREVIOUS
            # tile, interleaved between the current tile's matmul groups.
            pts, psbs, ptot, _ = pending
            cs = slice(p * PW, (p + 1) * PW)
            nc.vector.tensor_scalar_mul(psbs[p][:], psbs[p][:], ptot[:, 0:1])
            nc.sync.dma_start(d_out[pts, cs], psbs[p][:])

        pending = None
        prev_exp_insts = None
        for t in range(NT):
            ts = slice(t * 128, (t + 1) * 128)
            bias_ap = x2sb[:, t:t + 1]
            xh0t, xh1t = x_tiles[t]
            products = [(xh0t, wh0), (xh1t, wh1)]

            sfs = []
            sqrt_insts = []
            sums = sums_pool.tile([128, NP], F32, name=f"sums_{t}", tag="sums")
            # ---- phase S: matmul groups; DVE drain + in-place ACT sqrt
            for g in range(NG):
                if pending is not None and g == 2:
                    # reciprocal of the prev tile's row sums on gpsimd: keeps
                    # the DVE queue free of exp-gated work so psum drains are
                    # never FIFO-blocked behind it.
                    pts, psbs, ptot, pscr = pending
                    scr = sums_pool.tile([128, 1], F32, name=f"scr_{t}", tag="scr")
                    nc.gpsimd.normalize_recip(scr[:], pscr[:, 0:1], ptot[:])
                if pending is not None and 2 <= g < 2 + NP:
                    flush_one(pending, g - 2)
                ps = psum_pool.tile([128, GW], F32, name=f"ps_{t}_{g}", tag="ps")
                for p, (stat, mov) in enumerate(products):
                    for i in range(GRP):
                        cs = slice(g * GW + i * CH, g * GW + (i + 1) * CH)
                        nc.tensor.matmul(ps[:, i * CH:(i + 1) * CH],
                                         stat[:], mov[:, cs],
                                         start=(p == 0), stop=False)
                w2t = w2_tiles[(t, g)]
                for i in range(GRP):
                    nc.tensor.matmul(ps[:, i * CH:(i + 1) * CH],
                                     ones3[:, :], w2t[:, i * CH:(i + 1) * CH],
                                     start=False, stop=True)
                if g % 2 == 0:
                    sf = sf_pool.tile([128, PW], F32, name=f"sf_{t}_{g // 2}",
                                      tag="sf")
                    sfs.append(sf)
                nxt = t * NG + g + 2
                if nxt < NT * NG:
                    trig_w2(nxt // NG, nxt % NG)
                # DVE drains psum (+x2 bias): psum recycling must never wait
                # on the table-phase-ordered ACT queue, or the PE stalls each
                # tile and its p-state resets to 1.2 GHz.
                half = sfs[g // 2][:, (g % 2) * GW:(g % 2 + 1) * GW]
                nc.vector.tensor_scalar_add(half, ps[:], bias_ap)
                if g % 2 == 1:
                    sq = nc.scalar.activation(sfs[g // 2][:], sfs[g // 2][:],
                                              AF.Sqrt)
                    if prev_exp_insts is not None:
                        add_dep_helper(sq.ins, prev_exp_insts[-1].ins,
                                       reason="ACT phase order: sqrt after prev tile exps")
                    sqrt_insts.append(sq)
            pending = None
            # ---- phase E: 4096-wide exp -> bf16 slabs + accumulated row sums
            exp_insts = []
            sbs = []
            for p in range(NP):
                sb = sb_pool.tile([128, PW], BF16, name=f"sb_{t}_{p}", tag="sb")
                ex = nc.scalar.activation(sb[:], sfs[p][:], AF.Exp, scale=-2.0,
                                          accum_out=sums[:, p:p + 1])
                add_dep_helper(ex.ins, sqrt_insts[-1].ins,
                               reason="ACT phase order: exp after all sqrts in tile")
                exp_insts.append(ex)
                sbs.append(sb)
            # ---- x stationaries for the next tile (Sync, ahead of outs_t)
            if t + 1 < NT:
                load_x(t + 1)
            # ---- row-sum on ACT (tiny Identity w/ accumulate, stays inside
            # the exp table phase); reciprocal later via gpsimd
            scrN = sums_pool.tile([128, NP], F32, name=f"scrN_{t}", tag="scrN")
            tot = sums_pool.tile([128, 1], F32, name=f"tot_{t}", tag="tot")
            sum_act = nc.scalar.activation(scrN[:], sums[:], AF.Identity,
                                           accum_out=tot[:, 0:1])
            add_dep_helper(sum_act.ins, exp_insts[-1].ins,
                           reason="row-sum after exps on ACT")
            prev_exp_insts = [sum_act]
            pending = (ts, sbs, tot, scrN)

        if pending is not None:
            pts, psbs, ptot, pscr = pending
            scr = sums_pool.tile([128, 1], F32, name="scr_final", tag="scr")
            nc.gpsimd.normalize_recip(scr[:], pscr[:, 0:1], ptot[:])
            for p in range(NP):
                flush_one(pending, p)

    nc.compile()
    return nc


def _build(mode):
    if mode == "fast":
        return _build_fast()
    import concourse.bacc as bacc
    import concourse.tile as tile
    import concourse.mybir as mybir
    from concourse.tile import add_dep_helper

    F32 = mybir.dt.float32
    F32R = mybir.dt.float32r
    BF16 = mybir.dt.bfloat16
    AF = mybir.ActivationFunctionType

    nc = bacc.Bacc("TRN2", target_bir_lowering=False, debug=False,
                   num_devices=NCORES)

    if mode == "split7":
        d_wh = nc.dram_tensor("wh", [IN, OUT], BF16, kind="ExternalInput")
        d_wl = nc.dram_tensor("wl", [IN, OUT], BF16, kind="ExternalInput")
        d_xh = nc.dram_tensor("xh", [IN, BPC], BF16, kind="ExternalInput")
        d_xl = nc.dram_tensor("xl", [IN, BPC], BF16, kind="ExternalInput")
    else:
        d_wt = nc.dram_tensor("wt", [IN, OUT], F32R, kind="ExternalInput")
        d_xt = nc.dram_tensor("xt", [IN, BPC], F32R, kind="ExternalInput")
    d_w2s = nc.dram_tensor("w2s", [3, OUT], BF16, kind="ExternalInput")
    d_x2b = nc.dram_tensor("x2b", [128, NT], F32, kind="ExternalInput")
    d_out = nc.dram_tensor("out", [BPC, OUT], F32, kind="ExternalOutput")

    from contextlib import ExitStack
    with tile.TileContext(nc) as tc, ExitStack() as ctx:
        persist = ctx.enter_context(tc.tile_pool(name="persist", bufs=1))
        xt_pool = ctx.enter_context(tc.tile_pool(name="xtp", bufs=2))
        slab_pool = ctx.enter_context(tc.tile_pool(name="slabp", bufs=NG + 1))
        w2_pool = ctx.enter_context(tc.tile_pool(name="w2p", bufs=1))
        sums_pool = ctx.enter_context(tc.tile_pool(name="sumsp", bufs=2))
        psum_pool = ctx.enter_context(tc.tile_pool(name="psump", bufs=2, space="PSUM"))

        # ---- preload weights (split column-wise so early matmuls start early)
        if mode == "split7":
            wh0 = persist.tile([128, OUT], BF16, name="wh0")
            wh1 = persist.tile([128, OUT], BF16, name="wh1")
            wl0 = persist.tile([128, OUT], BF16, name="wl0")
            wl1 = persist.tile([128, OUT], BF16, name="wl1")
            wparts = [(wh0, d_wh, 0), (wh1, d_wh, 128), (wl0, d_wl, 0), (wl1, d_wl, 128)]
        else:
            wr0 = persist.tile([128, OUT], F32R, name="wr0")
            wr1 = persist.tile([128, OUT], F32R, name="wr1")
            wparts = [(wr0, d_wt, 0), (wr1, d_wt, 128)]
        NSPLIT = 8
        CW = OUT // NSPLIT
        for j in range(NSPLIT):
            cs = slice(j * CW, (j + 1) * CW)
            for t_sb, t_dram, p0 in wparts:
                nc.sync.dma_start(t_sb[:, cs], t_dram[p0:p0 + 128, cs])

        x2sb = persist.tile([128, NT], F32, name="x2sb")
        nc.sync.dma_start(x2sb[:], d_x2b[:, :])
        ones3 = persist.tile([3, 128], BF16, name="ones3")
        nc.vector.memset(ones3[:], 1.0)

        # x stationaries: tile t's slices are DMA'd during tile t-1 (t=0 in
        # preamble) on the Sync queue, BEFORE tile t-1's output DMAs are
        # emitted, so they never sit behind rec-gated outputs (FIFO HOL).
        x_tiles = {}

        def load_x(tt):
            ts_ = slice(tt * 128, (tt + 1) * 128)
            if mode == "split7":
                tiles = []
                for nm, dram, p0 in (("xh0t", d_xh, 0), ("xh1t", d_xh, 128),
                                     ("xl0t", d_xl, 0), ("xl1t", d_xl, 128)):
                    tl = xt_pool.tile([128, 128], BF16, name=f"{nm}_{tt}", tag=nm)
                    nc.sync.dma_start(tl[:], dram[p0:p0 + 128, ts_])
                    tiles.append(tl)
            else:
                tiles = []
                for nm, p0 in (("xr0t", 0), ("xr1t", 128)):
                    tl = xt_pool.tile([128, 128], F32R, name=f"{nm}_{tt}", tag=nm)
                    nc.sync.dma_start(tl[:], d_xt[p0:p0 + 128, ts_])
                    tiles.append(tl)
            x_tiles[tt] = tiles

        load_x(0)

        # w2 chunk tiles (single slot): group g+1's DMA is triggered from the
        # ACT queue right before group g's sqrt, matching slot-free timing.
        w2_tiles = {}

        def trig_w2(tt, gg):
            # gpsimd SWDGE: separate queue + semaphore space from the HWDGE
            # queues that carry the (rec-gated) output DMAs, so the K=3
            # matmul's wait on this DMA never counts late output completions.
            w2t = w2_pool.tile([3, GW], BF16, name=f"w2t_{tt}_{gg}", tag="w2t")
            ins = nc.gpsimd.dma_start(w2t[:], d_w2s[:, gg * GW:(gg + 1) * GW])
            w2_tiles[(tt, gg)] = w2t
            return ins

        trig_w2(0, 0)

        def flush_one(pending, g):
            # normalize (DVE) + store (Sync) of ONE chunk of the PREVIOUS
            # tile; interleaved between the current tile's matmul groups so
            # next-tile psum drains are never FIFO-blocked behind a full
            # batch of rec-gated normalizes.
            pts, pslabs, ptot = pending[0], pending[1], pending[2]
            gs = slice(g * GW, (g + 1) * GW)
            nc.vector.tensor_scalar_mul(pslabs[g][:], pslabs[g][:], ptot[:, 0:1])
            nc.sync.dma_start(d_out[pts, gs], pslabs[g][:])

        pending = None
        prev_exp_insts = None
        for t in range(NT):
            ts = slice(t * 128, (t + 1) * 128)
            bias_ap = x2sb[:, t:t + 1]
            if mode == "split7":
                xh0t, xh1t, xl0t, xl1t = x_tiles[t]
                products = [(xh0t, wh0), (xh0t, wl0), (xl0t, wh0),
                            (xh1t, wh1), (xh1t, wl1), (xl1t, wh1)]
            else:
                xr0t, xr1t = x_tiles[t]
                products = [(xr0t, wr0), (xr1t, wr1)]

            slabs = []
            sqrt_insts = []
            sums = sums_pool.tile([128, NG], F32, name=f"sums_{t}", tag="sums")
            # ---- phase S: matmul groups + 2048-wide sqrt
            for g in range(NG):
                if pending is not None and g == 2:
                    # reciprocal of the prev tile's row sums: emitted after
                    # this tile's first w2 triggers so they are not blocked
                    # behind it in the gpsimd FIFO
                    pts, pslabs, ptot, pscr8 = pending
                    scr = sums_pool.tile([128, 1], F32, name=f"scr_{t}", tag="scr")
                    nc.gpsimd.normalize_recip(scr[:], pscr8[:, 0:1], ptot[:])
                if pending is not None and g >= 2:
                    flush_one(pending, g - 2)
                ps = psum_pool.tile([128, GW], F32, name=f"ps_{t}_{g}", tag="ps")
                # product-major: stationary reused across the GRP sub-chunks
                for p, (stat, mov) in enumerate(products):
                    for i in range(GRP):
                        cs = slice(g * GW + i * CH, g * GW + (i + 1) * CH)
                        nc.tensor.matmul(ps[:, i * CH:(i + 1) * CH],
                                         stat[:], mov[:, cs],
                                         start=(p == 0), stop=False)
                w2t = w2_tiles[(t, g)]
                for i in range(GRP):
                    nc.tensor.matmul(ps[:, i * CH:(i + 1) * CH],
                                     ones3[:, :], w2t[:, i * CH:(i + 1) * CH],
                                     start=False, stop=True)
                # DVE drains psum (and adds the x2 bias): the DVE queue holds
                # ONLY drains, so psum recycling never stalls behind rec-gated
                # work; slab pool is the PE runway.
                sl = slab_pool.tile([128, GW], F32, name=f"slab_{t}_{g}", tag="slab")
                nc.vector.tensor_scalar_add(sl[:], ps[:], bias_ap)
                # trigger the next group's w2 DMA (gpsimd queue)
                if (t, g) != (NT - 1, NG - 1):
                    nt_, ng_ = (t, g + 1) if g + 1 < NG else (t + 1, 0)
                    trig_w2(nt_, ng_)
                sq = nc.scalar.activation(sl[:], sl[:], AF.Sqrt)
                if prev_exp_insts is not None:
                    add_dep_helper(sq.ins, prev_exp_insts[-1].ins,
                                   reason="ACT phase order: sqrt after prev tile exps")
                slabs.append(sl)
                sqrt_insts.append(sq)
            if pending is not None:
                flush_one(pending, NG - 2)
                flush_one(pending, NG - 1)
                pending = None
            # ---- phase E: 2048-wide exp with accumulated row sums
            exp_insts = []
            for g in range(NG):
                ex = nc.scalar.activation(slabs[g][:], slabs[g][:], AF.Exp,
                                          scale=-2.0, accum_out=sums[:, g:g + 1])
                add_dep_helper(ex.ins, sqrt_insts[-1].ins,
                               reason="ACT phase order: exp after all sqrts in tile")
                exp_insts.append(ex)
            # ---- x stationaries for the next tile (Sync, ahead of outs_t)
            if t + 1 < NT:
                load_x(t + 1)
            # ---- row-sum on ACT (tiny Identity w/ accumulate; stays inside
            # the exp phase), reciprocal via gpsimd normalize_recip (the only
            # gpsimd op, so its library stays loaded)
            scr8 = sums_pool.tile([128, NG], F32, name=f"scr8_{t}", tag="scr8")
            tot = sums_pool.tile([128, 1], F32, name=f"tot_{t}", tag="tot")
            sum_act = nc.scalar.activation(scr8[:], sums[:], AF.Identity,
                                           accum_out=tot[:, 0:1])
            add_dep_helper(sum_act.ins, exp_insts[-1].ins,
                           reason="row-sum after exps on ACT")
            prev_exp_insts = [sum_act]
            pending = (ts, slabs, tot, scr8)

        if pending is not None:
            pts, pslabs, ptot, pscr8 = pending
            scr = sums_pool.tile([128, 1], F32, name="scr_final", tag="scr")
            nc.gpsimd.normalize_recip(scr[:], pscr8[:, 0:1], ptot[:])
            for g in range(NG):
                flush_one(pending, g)

    nc.compile()
    return nc


def _get_nc(mode):
    if mode not in _built:
        _built[mode] = _build(mode)
    return _built[mode]


def _prep_inputs(x, weight, mode):
    x = np.ascontiguousarray(np.asarray(x, dtype=np.float32))
    weight = np.ascontiguousarray(np.asarray(weight, dtype=np.float32))
    assert x.shape == (B, IN) and weight.shape == (OUT, IN)

    wt = np.ascontiguousarray(weight.T).astype(np.float32)       # [IN, OUT]
    w2 = np.sum(weight.astype(np.float64) ** 2, axis=1)
    w2c = (w2 - 256.0).astype(np.float32)
    w2a = w2c.astype(_BF16)
    r1 = w2c - w2a.astype(np.float32)
    w2b = r1.astype(_BF16)
    w2d = (r1 - w2b.astype(np.float32)).astype(_BF16)
    w2s = np.ascontiguousarray(np.stack([w2a, w2b, w2d], axis=0))  # [3, OUT]

    shared = {"w2s": w2s}
    if mode == "split7":
        wh, wl = _bf16_split(wt)
        shared["wh"] = wh
        shared["wl"] = wl
    elif mode == "fast":
        shared["wt"] = wt.astype(np.float16)
    else:
        shared["wt"] = wt  # raw fp32 bits, declared float32r on device

    in_maps = []
    for i in range(NCORES):
        xs = x[i * BPC:(i + 1) * BPC]                             # [BPC, IN]
        xt = np.ascontiguousarray((-2.0 * xs.T).astype(np.float32))  # [IN, BPC]
        x2 = np.sum(xs.astype(np.float64) ** 2, axis=1).astype(np.float32) + 256.0
        x2b = np.ascontiguousarray(x2.reshape(NT, 128).T).astype(np.float32)
        m = dict(shared)
        if mode == "split7":
            xh, xl = _bf16_split(xt)
            m["xh"] = xh
            m["xl"] = xl
        elif mode == "fast":
            m["xt"] = xt.astype(np.float16)
        else:
            m["xt"] = xt
        m["x2b"] = x2b
        in_maps.append(m)
    return in_maps


def _run(x, weight, mode=None, trace=False, trace_cores=None):
    from concourse.bass_utils import run_bass_kernel_spmd
    mode = mode or MODE
    nc = _get_nc(mode)
    in_maps = _prep_inputs(x, weight, mode)
    res = run_bass_kernel_spmd(nc, in_maps, list(range(NCORES)), trace=trace,
                               trace_cores=trace_cores)
    out = np.concatenate([np.asarray(res.results[i]["out"], dtype=np.float32)
                          for i in range(NCORES)], axis=0)
    return out, res


def kernel(x, weight):
    out, _ = _run(x, weight)
    return out


def kernel_profiled(x, weight, mode=None, trace_cores=None):
    """Returns (out, exec_time_ns, trace_path)."""
    out, res = _run(x, weight, mode=mode, trace=True, trace_cores=trace_cores)
    trace_path = None
    if res.instructions_and_trace is not None:
        trace_path = res.instructions_and_trace[1]
    return out, res.exec_time_ns, trace_path



# revision 21
# speedup vs baseline: 1.0315x; 1.0315x over previous
"""TRN2 Bass kernel for nn_MetrixSoftmax: softmax(-2 * ||x_b - w_o||_2, axis=o).

x: [8192, 256] f32, weight: [16384, 256] f32 -> out: [8192, 16384] f32.

Strategy: data-parallel shard x over batch across 8 cores (1024 rows each),
replicate weight; each core computes its full output rows so the softmax
needs no collectives. Per core (layout: partitions=batch rows, free=out):

  d2[b,o] = (x2[b]+256) + (w2[o]-256) - 2*x.w
  psum    = matmul-accumulated [-2*x.w + (w2-256)]  (w2 row folded in as a
            K=3 bf16 matmul of an exact 3-way bf16 split of w2-256)
  dist    = ACT Sqrt(psum + bias[b])   (bias = x2+256, per-partition AP)
  e       = ACT Exp(-2*dist), accum_out -> per-row partial sums
  out     = e * (1/sum)                (DVE reciprocal + tensor_scalar_mul)

Matmul modes:
  split7: x and w.T each split hi/lo bf16; 6 bf16 matmuls (hh, hl, lh) +
          w2 row -> ~7e-4 abs err on d2 (fp32-grade output).
  f32r3:  x, w.T fed as raw fp32 bits declared float32r (tf32-like 1-pass);
          2 matmuls + w2 row -> ~4e-2 abs err on d2 (~1e-3 output rel err).

Performance structure: chunks of G=4 psum banks [128, 2048]; matmuls run
product-major inside a group so the PE stationary operand is reused 4x;
ACT processes 2048-wide chunks; sqrt/exp phases are strictly ordered per
batch-tile to get exactly 2 ACT table loads per tile.
"""

import numpy as np
import ml_dtypes

B, IN, OUT = 8192, 256, 16384
NCORES = 8
BPC = B // NCORES     # 1024 batch rows per core
NT = BPC // 128       # 8 batch tiles of 128 rows
CH = 512              # matmul free-dim (one PSUM bank)
GRP = 4               # chunks per psum/slab group
GW = CH * GRP         # 2048 group width
NG = OUT // GW        # 8 groups per batch tile

MODE = "split7"       # "split7" (accurate) | "f32r3" | "fast" (fp16+bf16out)

_BF16 = ml_dtypes.bfloat16
_built = {}


def _bf16_split(a):
    hi = a.astype(_BF16)
    lo = (a - hi.astype(np.float32)).astype(_BF16)
    return hi, lo


def _build_fast():
    """fp16-matmul pipeline tuned for the ACT (scalar-engine) roofline.

    Per core: psum = [-2x.w + (w2-256)] via 2 fp16 K=128 matmuls + K=3 bf16
    w2 row (fp16 input quantization -> ~0.06 abs err on d2, ~3.5e-3 output
    scale_rel incl bf16 output rounding). ACT Sqrt reads PSUM directly with
    per-partition bias (x2+256) -- no DVE drain pass. ACT Exp (scale=-2)
    writes bf16 slabs 4096-wide with accum_out row sums. DVE does only the
    [128,4]->[128,1] sum reduce, reciprocal, and the 2x-mode bf16 normalize
    mul. Output DMA'd as bf16, upcast to f32 on host.

    Engine budget per core: ACT ~260us (bottleneck: sqrt 118 + exp 121 +
    16 table loads 20), PE ~170us, DVE ~50us, DMA ~42MB ~ 125us.
    """
    import concourse.bacc as bacc
    import concourse.tile as tile
    import concourse.mybir as mybir
    from concourse.tile import add_dep_helper

    F32 = mybir.dt.float32
    F16 = mybir.dt.float16
    BF16 = mybir.dt.bfloat16
    AF = mybir.ActivationFunctionType
    PW = 2 * GW           # 4096: exp/normalize/store pair width
    NP = OUT // PW        # 4 pairs per batch tile

    nc = bacc.Bacc("TRN2", target_bir_lowering=False, debug=False,
                   num_devices=NCORES)

    d_wt = nc.dram_tensor("wt", [IN, OUT], F16, kind="ExternalInput")
    d_xt = nc.dram_tensor("xt", [IN, BPC], F16, kind="ExternalInput")
    d_w2s = nc.dram_tensor("w2s", [3, OUT], BF16, kind="ExternalInput")
    d_x2b = nc.dram_tensor("x2b", [128, NT], F32, kind="ExternalInput")
    d_out = nc.dram_tensor("out", [BPC, OUT], BF16, kind="ExternalOutput")

    from contextlib import ExitStack
    with tile.TileContext(nc) as tc, ExitStack() as ctx:
        persist = ctx.enter_context(tc.tile_pool(name="persist", bufs=1))
        xt_pool = ctx.enter_context(tc.tile_pool(name="xtp", bufs=2))
        sf_pool = ctx.enter_context(tc.tile_pool(name="sfp", bufs=NP + 1))
        sb_pool = ctx.enter_context(tc.tile_pool(name="sbp", bufs=NP + 1))
        w2_pool = ctx.enter_context(tc.tile_pool(name="w2p", bufs=3))
        sums_pool = ctx.enter_context(tc.tile_pool(name="sumsp", bufs=2))
        psum_pool = ctx.enter_context(tc.tile_pool(name="psump", bufs=2, space="PSUM"))

        wh0 = persist.tile([128, OUT], F16, name="wh0")
        wh1 = persist.tile([128, OUT], F16, name="wh1")
        x2sb = persist.tile([128, NT], F32, name="x2sb")
        ones3 = persist.tile([3, 128], BF16, name="ones3")
        nc.vector.memset(ones3[:], 1.0)
        dummy_m = persist.tile([3, 512], BF16, name="dummy_m")
        nc.vector.memset(dummy_m[:], 0.0)

        x_tiles = {}

        def load_x(tt):
            ts_ = slice(tt * 128, (tt + 1) * 128)
            tiles = []
            for nm, p0 in (("xh0t", 0), ("xh1t", 128)):
                tl = xt_pool.tile([128, 128], F16, name=f"{nm}_{tt}", tag=nm)
                nc.sync.dma_start(tl[:], d_xt[p0:p0 + 128, ts_])
                tiles.append(tl)
            x_tiles[tt] = tiles

        # x/x2 first: they are tiny and the first matmul needs them; the 8.4MB
        # weight preload must not sit ahead of them in the Sync DMA queue.
        load_x(0)
        nc.sync.dma_start(x2sb[:], d_x2b[:, :])

        # ---- preload weights (column-split so early matmuls start early)
        NSPLIT = 8
        CW = OUT // NSPLIT
        for j in range(NSPLIT):
            cs = slice(j * CW, (j + 1) * CW)
            nc.sync.dma_start(wh0[:, cs], d_wt[0:128, cs])
            nc.sync.dma_start(wh1[:, cs], d_wt[128:256, cs])

        # ---- PE pre-heat: dummy matmuls keep the PE continuously busy while
        # the first weight/x DMAs land, so the p-state is fully ramped
        # (2.4 GHz) when real groups start; group 0's start=True resets psum.
        ps_warm = psum_pool.tile([128, GW], F32, name="ps_warm", tag="ps")
        for k in range(9):
            nc.tensor.matmul(ps_warm[:, 0:512], ones3[:, :], dummy_m[:],
                             start=(k == 0), stop=(k == 8))

        # w2 chunk tiles on the gpsimd SWDGE queue (separate from the
        # rec-gated output DMAs on the Sync queues).
        w2_tiles = {}

        def trig_w2(tt, gg):
            w2t = w2_pool.tile([3, GW], BF16, name=f"w2t_{tt}_{gg}", tag="w2t")
            nc.gpsimd.dma_start(w2t[:], d_w2s[:, gg * GW:(gg + 1) * GW])
            w2_tiles[(tt, gg)] = w2t

        # triple-buffered, triggered 2 groups ahead: the SWDGE trigger + DMA
        # latency (~2-3us) must never sit in the PE's path between groups.
        trig_w2(0, 0)
        trig_w2(0, 1)

        def flush_one(pending, p):
            # normalize (DVE 2x bf16) + store (Sync) ONE pair of the PREVIOUS
            # tile, interleaved between the current tile's matmul groups.
            pts, psbs, ptot, _ = pending
            cs = slice(p * PW, (p + 1) * PW)
            nc.vector.tensor_scalar_mul(psbs[p][:], psbs[p][:], ptot[:, 0:1])
            nc.sync.dma_start(d_out[pts, cs], psbs[p][:])

        pending = None
        prev_exp_insts = None
        for t in range(NT):
            ts = slice(t * 128, (t + 1) * 128)
            bias_ap = x2sb[:, t:t + 1]
            xh0t, xh1t = x_tiles[t]
            products = [(xh0t, wh0), (xh1t, wh1)]

            sfs = []
            sqrt_insts = []
            sums = sums_pool.tile([128, NP], F32, name=f"sums_{t}", tag="sums")
            # ---- phase S: matmul groups; DVE drain + in-place ACT sqrt
            for g in range(NG):
                if pending is not None and g == 2:
                    # reciprocal of the prev tile's row sums on gpsimd: keeps
                    # the DVE queue free of exp-gated work so psum drains are
                    # never FIFO-blocked behind it.
                    pts, psbs, ptot, pscr = pending
                    scr = sums_pool.tile([128, 1], F32, name=f"scr_{t}", tag="scr")
                    nc.gpsimd.normalize_recip(scr[:], pscr[:, 0:1], ptot[:])
                if pending is not None and 2 <= g < 2 + NP:
                    flush_one(pending, g - 2)
                ps = psum_pool.tile([128, GW], F32, name=f"ps_{t}_{g}", tag="ps")
                for p, (stat, mov) in enumerate(products):
                    for i in range(GRP):
                        cs = slice(g * GW + i * CH, g * GW + (i + 1) * CH)
                        nc.tensor.matmul(ps[:, i * CH:(i + 1) * CH],
                                         stat[:], mov[:, cs],
                                         start=(p == 0), stop=False)
                w2t = w2_tiles[(t, g)]
                for i in range(GRP):
                    nc.tensor.matmul(ps[:, i * CH:(i + 1) * CH],
                                     ones3[:, :], w2t[:, i * CH:(i + 1) * CH],
                                     start=False, stop=True)
                if g % 2 == 0:
                    sf = sf_pool.tile([128, PW], F32, name=f"sf_{t}_{g // 2}",
                                      tag="sf")
                    sfs.append(sf)
                nxt = t * NG + g + 2
                if nxt < NT * NG:
                    trig_w2(nxt // NG, nxt % NG)
                # DVE drains psum (+x2 bias): psum recycling must never wait
                # on the table-phase-ordered ACT queue, or the PE stalls each
                # tile and its p-state resets to 1.2 GHz.
                half = sfs[g // 2][:, (g % 2) * GW:(g % 2 + 1) * GW]
                nc.vector.tensor_scalar_add(half, ps[:], bias_ap)
                if t == 0:
                    # tile 0 is PE-warmup-paced: narrow sqrts track the drains
                    # so phase E starts one drain earlier
                    sqrt_insts.append(nc.scalar.activation(half, half, AF.Sqrt))
                elif g % 2 == 1:
                    sq = nc.scalar.activation(sfs[g // 2][:], sfs[g // 2][:],
                                              AF.Sqrt)
                    add_dep_helper(sq.ins, prev_exp_insts[-1].ins,
                                   reason="ACT phase order: sqrt after prev tile exps")
                    sqrt_insts.append(sq)
            pending = None
            # ---- phase E: 4096-wide exp -> bf16 slabs + accumulated row sums
            exp_insts = []
            sbs = []
            for p in range(NP):
                sb = sb_pool.tile([128, PW], BF16, name=f"sb_{t}_{p}", tag="sb")
                ex = nc.scalar.activation(sb[:], sfs[p][:], AF.Exp, scale=-2.0,
                                          accum_out=sums[:, p:p + 1])
                add_dep_helper(ex.ins, sqrt_insts[-1].ins,
                               reason="ACT phase order: exp after all sqrts in tile")
                exp_insts.append(ex)
                sbs.append(sb)
            # ---- x stationaries for the next tile (Sync, ahead of outs_t)
            if t + 1 < NT:
                load_x(t + 1)
            # ---- row-sum on ACT (tiny Identity w/ accumulate, stays inside
            # the exp table phase); reciprocal later via gpsimd
            scrN = sums_pool.tile([128, NP], F32, name=f"scrN_{t}", tag="scrN")
            tot = sums_pool.tile([128, 1], F32, name=f"tot_{t}", tag="tot")
            sum_act = nc.scalar.activation(scrN[:], sums[:], AF.Identity,
                                           accum_out=tot[:, 0:1])
            add_dep_helper(sum_act.ins, exp_insts[-1].ins,
                           reason="row-sum after exps on ACT")
            prev_exp_insts = [sum_act]
            pending = (ts, sbs, tot, scrN)

        if pending is not None:
            # tail: fine-grained flush; odd chunks go out on the Activation
            # HWDGE queue (ACT is drained by now) to halve queue serialization
            pts, psbs, ptot, pscr = pending
            scr = sums_pool.tile([128, 1], F32, name="scr_final", tag="scr")
            nc.gpsimd.normalize_recip(scr[:], pscr[:, 0:1], ptot[:])
            for c in range(2 * NP):
                p, h = c // 2, c % 2
                hs = slice(h * GW, (h + 1) * GW)
                nc.vector.tensor_scalar_mul(psbs[p][:, hs], psbs[p][:, hs],
                                            ptot[:, 0:1])
                cs = slice(p * PW + h * GW, p * PW + (h + 1) * GW)
                eng = nc.sync if c % 2 == 0 else nc.scalar
                eng.dma_start(d_out[pts, cs], psbs[p][:, hs])

    nc.compile()
    return nc


def _build(mode):
    if mode == "fast":
        return _build_fast()
    import concourse.bacc as bacc
    import concourse.tile as tile
    import concourse.mybir as mybir
    from concourse.tile import add_dep_helper

    F32 = mybir.dt.float32
    F32R = mybir.dt.float32r
    BF16 = mybir.dt.bfloat16
    AF = mybir.ActivationFunctionType

    nc = bacc.Bacc("TRN2", target_bir_lowering=False, debug=False,
                   num_devices=NCORES)

    if mode == "split7":
        d_wh = nc.dram_tensor("wh", [IN, OUT], BF16, kind="ExternalInput")
        d_wl = nc.dram_tensor("wl", [IN, OUT], BF16, kind="ExternalInput")
        d_xh = nc.dram_tensor("xh", [IN, BPC], BF16, kind="ExternalInput")
        d_xl = nc.dram_tensor("xl", [IN, BPC], BF16, kind="ExternalInput")
    else:
        d_wt = nc.dram_tensor("wt", [IN, OUT], F32R, kind="ExternalInput")
        d_xt = nc.dram_tensor("xt", [IN, BPC], F32R, kind="ExternalInput")
    d_w2s = nc.dram_tensor("w2s", [3, OUT], BF16, kind="ExternalInput")
    d_x2b = nc.dram_tensor("x2b", [128, NT], F32, kind="ExternalInput")
    d_out = nc.dram_tensor("out", [BPC, OUT], F32, kind="ExternalOutput")

    from contextlib import ExitStack
    with tile.TileContext(nc) as tc, ExitStack() as ctx:
        persist = ctx.enter_context(tc.tile_pool(name="persist", bufs=1))
        xt_pool = ctx.enter_context(tc.tile_pool(name="xtp", bufs=2))
        slab_pool = ctx.enter_context(tc.tile_pool(name="slabp", bufs=NG + 1))
        w2_pool = ctx.enter_context(tc.tile_pool(name="w2p", bufs=1))
        sums_pool = ctx.enter_context(tc.tile_pool(name="sumsp", bufs=2))
        psum_pool = ctx.enter_context(tc.tile_pool(name="psump", bufs=2, space="PSUM"))

        # ---- preload weights (split column-wise so early matmuls start early)
        if mode == "split7":
            wh0 = persist.tile([128, OUT], BF16, name="wh0")
            wh1 = persist.tile([128, OUT], BF16, name="wh1")
            wl0 = persist.tile([128, OUT], BF16, name="wl0")
            wl1 = persist.tile([128, OUT], BF16, name="wl1")
            wparts = [(wh0, d_wh, 0), (wh1, d_wh, 128), (wl0, d_wl, 0), (wl1, d_wl, 128)]
        else:
            wr0 = persist.tile([128, OUT], F32R, name="wr0")
            wr1 = persist.tile([128, OUT], F32R, name="wr1")
            wparts = [(wr0, d_wt, 0), (wr1, d_wt, 128)]
        NSPLIT = 8
        CW = OUT // NSPLIT
        for j in range(NSPLIT):
            cs = slice(j * CW, (j + 1) * CW)
            for t_sb, t_dram, p0 in wparts:
                nc.sync.dma_start(t_sb[:, cs], t_dram[p0:p0 + 128, cs])

        x2sb = persist.tile([128, NT], F32, name="x2sb")
        nc.sync.dma_start(x2sb[:], d_x2b[:, :])
        ones3 = persist.tile([3, 128], BF16, name="ones3")
        nc.vector.memset(ones3[:], 1.0)

        # x stationaries: tile t's slices are DMA'd during tile t-1 (t=0 in
        # preamble) on the Sync queue, BEFORE tile t-1's output DMAs are
        # emitted, so they never sit behind rec-gated outputs (FIFO HOL).
        x_tiles = {}

        def load_x(tt):
            ts_ = slice(tt * 128, (tt + 1) * 128)
            if mode == "split7":
                tiles = []
                for nm, dram, p0 in (("xh0t", d_xh, 0), ("xh1t", d_xh, 128),
                                     ("xl0t", d_xl, 0), ("xl1t", d_xl, 128)):
                    tl = xt_pool.tile([128, 128], BF16, name=f"{nm}_{tt}", tag=nm)
                    nc.sync.dma_start(tl[:], dram[p0:p0 + 128, ts_])
                    tiles.append(tl)
            else:
                tiles = []
                for nm, p0 in (("xr0t", 0), ("xr1t", 128)):
                    tl = xt_pool.tile([128, 128], F32R, name=f"{nm}_{tt}", tag=nm)
                    nc.sync.dma_start(tl[:], d_xt[p0:p0 + 128, ts_])
                    tiles.append(tl)
            x_tiles[tt] = tiles

        load_x(0)

        # w2 chunk tiles (single slot): group g+1's DMA is triggered from the
        # ACT queue right before group g's sqrt, matching slot-free timing.
        w2_tiles = {}

        def trig_w2(tt, gg):
            # gpsimd SWDGE: separate queue + semaphore space from the HWDGE
            # queues that carry the (rec-gated) output DMAs, so the K=3
            # matmul's wait on this DMA never counts late output completions.
            w2t = w2_pool.tile([3, GW], BF16, name=f"w2t_{tt}_{gg}", tag="w2t")
            ins = nc.gpsimd.dma_start(w2t[:], d_w2s[:, gg * GW:(gg + 1) * GW])
            w2_tiles[(tt, gg)] = w2t
            return ins

        trig_w2(0, 0)

        def flush_one(pending, g):
            # normalize (DVE) + store (Sync) of ONE chunk of the PREVIOUS
            # tile; interleaved between the current tile's matmul groups so
            # next-tile psum drains are never FIFO-blocked behind a full
            # batch of rec-gated normalizes.
            pts, pslabs, ptot = pending[0], pending[1], pending[2]
            gs = slice(g * GW, (g + 1) * GW)
            nc.vector.tensor_scalar_mul(pslabs[g][:], pslabs[g][:], ptot[:, 0:1])
            nc.sync.dma_start(d_out[pts, gs], pslabs[g][:])

        pending = None
        prev_exp_insts = None
        for t in range(NT):
            ts = slice(t * 128, (t + 1) * 128)
            bias_ap = x2sb[:, t:t + 1]
            if mode == "split7":
                xh0t, xh1t, xl0t, xl1t = x_tiles[t]
                products = [(xh0t, wh0), (xh0t, wl0), (xl0t, wh0),
                            (xh1t, wh1), (xh1t, wl1), (xl1t, wh1)]
            else:
                xr0t, xr1t = x_tiles[t]
                products = [(xr0t, wr0), (xr1t, wr1)]

            slabs = []
            sqrt_insts = []
            sums = sums_pool.tile([128, NG], F32, name=f"sums_{t}", tag="sums")
            # ---- phase S: matmul groups + 2048-wide sqrt
            for g in range(NG):
                if pending is not None and g == 2:
                    # reciprocal of the prev tile's row sums: emitted after
                    # this tile's first w2 triggers so they are not blocked
                    # behind it in the gpsimd FIFO
                    pts, pslabs, ptot, pscr8 = pending
                    scr = sums_pool.tile([128, 1], F32, name=f"scr_{t}", tag="scr")
                    nc.gpsimd.normalize_recip(scr[:], pscr8[:, 0:1], ptot[:])
                if pending is not None and g >= 2:
                    flush_one(pending, g - 2)
                ps = psum_pool.tile([128, GW], F32, name=f"ps_{t}_{g}", tag="ps")
                # product-major: stationary reused across the GRP sub-chunks
                for p, (stat, mov) in enumerate(products):
                    for i in range(GRP):
                        cs = slice(g * GW + i * CH, g * GW + (i + 1) * CH)
                        nc.tensor.matmul(ps[:, i * CH:(i + 1) * CH],
                                         stat[:], mov[:, cs],
                                         start=(p == 0), stop=False)
                w2t = w2_tiles[(t, g)]
                for i in range(GRP):
                    nc.tensor.matmul(ps[:, i * CH:(i + 1) * CH],
                                     ones3[:, :], w2t[:, i * CH:(i + 1) * CH],
                                     start=False, stop=True)
                # DVE drains psum (and adds the x2 bias): the DVE queue holds
                # ONLY drains, so psum recycling never stalls behind rec-gated
                # work; slab pool is the PE runway.
                sl = slab_pool.tile([128, GW], F32, name=f"slab_{t}_{g}", tag="slab")
                nc.vector.tensor_scalar_add(sl[:], ps[:], bias_ap)
                # trigger the next group's w2 DMA (gpsimd queue)
                if (t, g) != (NT - 1, NG - 1):
                    nt_, ng_ = (t, g + 1) if g + 1 < NG else (t + 1, 0)
                    trig_w2(nt_, ng_)
                sq = nc.scalar.activation(sl[:], sl[:], AF.Sqrt)
                if prev_exp_insts is not None:
                    add_dep_helper(sq.ins, prev_exp_insts[-1].ins,
                                   reason="ACT phase order: sqrt after prev tile exps")
                slabs.append(sl)
                sqrt_insts.append(sq)
            if pending is not None:
                flush_one(pending, NG - 2)
                flush_one(pending, NG - 1)
                pending = None
            # ---- phase E: 2048-wide exp with accumulated row sums
            exp_insts = []
            for g in range(NG):
                ex = nc.scalar.activation(slabs[g][:], slabs[g][:], AF.Exp,
                                          scale=-2.0, accum_out=sums[:, g:g + 1])
                add_dep_helper(ex.ins, sqrt_insts[-1].ins,
                               reason="ACT phase order: exp after all sqrts in tile")
                exp_insts.append(ex)
            # ---- x stationaries for the next tile (Sync, ahead of outs_t)
            if t + 1 < NT:
                load_x(t + 1)
            # ---- row-sum on ACT (tiny Identity w/ accumulate; stays inside
            # the exp phase), reciprocal via gpsimd normalize_recip (the only
            # gpsimd op, so its library stays loaded)
            scr8 = sums_pool.tile([128, NG], F32, name=f"scr8_{t}", tag="scr8")
            tot = sums_pool.tile([128, 1], F32, name=f"tot_{t}", tag="tot")
            sum_act = nc.scalar.activation(scr8[:], sums[:], AF.Identity,
                                           accum_out=tot[:, 0:1])
            add_dep_helper(sum_act.ins, exp_insts[-1].ins,
                           reason="row-sum after exps on ACT")
            prev_exp_insts = [sum_act]
            pending = (ts, slabs, tot, scr8)

        if pending is not None:
            pts, pslabs, ptot, pscr8 = pending
            scr = sums_pool.tile([128, 1], F32, name="scr_final", tag="scr")
            nc.gpsimd.normalize_recip(scr[:], pscr8[:, 0:1], ptot[:])
            for g in range(NG):
                flush_one(pending, g)

    nc.compile()
    return nc


def _get_nc(mode):
    if mode not in _built:
        _built[mode] = _build(mode)
    return _built[mode]


def _prep_inputs(x, weight, mode):
    x = np.ascontiguousarray(np.asarray(x, dtype=np.float32))
    weight = np.ascontiguousarray(np.asarray(weight, dtype=np.float32))
    assert x.shape == (B, IN) and weight.shape == (OUT, IN)

    wt = np.ascontiguousarray(weight.T).astype(np.float32)       # [IN, OUT]
    w2 = np.sum(weight.astype(np.float64) ** 2, axis=1)
    w2c = (w2 - 256.0).astype(np.float32)
    w2a = w2c.astype(_BF16)
    r1 = w2c - w2a.astype(np.float32)
    w2b = r1.astype(_BF16)
    w2d = (r1 - w2b.astype(np.float32)).astype(_BF16)
    w2s = np.ascontiguousarray(np.stack([w2a, w2b, w2d], axis=0))  # [3, OUT]

    shared = {"w2s": w2s}
    if mode == "split7":
        wh, wl = _bf16_split(wt)
        shared["wh"] = wh
        shared["wl"] = wl
    elif mode == "fast":
        shared["wt"] = wt.astype(np.float16)
    else:
        shared["wt"] = wt  # raw fp32 bits, declared float32r on device

    in_maps = []
    for i in range(NCORES):
        xs = x[i * BPC:(i + 1) * BPC]                             # [BPC, IN]
        xt = np.ascontiguousarray((-2.0 * xs.T).astype(np.float32))  # [IN, BPC]
        x2 = np.sum(xs.astype(np.float64) ** 2, axis=1).astype(np.float32) + 256.0
        x2b = np.ascontiguousarray(x2.reshape(NT, 128).T).astype(np.float32)
        m = dict(shared)
        if mode == "split7":
            xh, xl = _bf16_split(xt)
            m["xh"] = xh
            m["xl"] = xl
        elif mode == "fast":
            m["xt"] = xt.astype(np.float16)
        else:
            m["xt"] = xt
        m["x2b"] = x2b
        in_maps.append(m)
    return in_maps


def _run(x, weight, mode=None, trace=False, trace_cores=None):
    from concourse.bass_utils import run_bass_kernel_spmd
    mode = mode or MODE
    nc = _get_nc(mode)
    in_maps = _prep_inputs(x, weight, mode)
    res = run_bass_kernel_spmd(nc, in_maps, list(range(NCORES)), trace=trace,
                               trace_cores=trace_cores)
    out = np.concatenate([np.asarray(res.results[i]["out"], dtype=np.float32)
                          for i in range(NCORES)], axis=0)
    return out, res


def kernel(x, weight):
    out, _ = _run(x, weight)
    return out


def kernel_profiled(x, weight, mode=None, trace_cores=None):
    """Returns (out, exec_time_ns, trace_path)."""
    out, res = _run(x, weight, mode=mode, trace=True, trace_cores=trace_cores)
    trace_path = None
    if res.instructions_and_trace is not None:
        trace_path = res.instructions_and_trace[1]
    return out, res.exec_time_ns, trace_path

